# revision 1
# baseline (speedup 1.0000x reference)
"""Trainium2 Bass kernel for nn_ColorHistograms.

Pipeline (per NeuronCore, 2 batch elements each, 8 cores):
  1. Stream x tiles [128 frames, 3888] from HBM in 3 chunk-DMAs each (spreads
     load over DMA engines; one engine tops out at ~22.5 GB/s). Per-channel
     spatial means: channels 0,1 on ScalarE (activation Copy + accum_out),
     channel 2 on VectorE (strided tensor_reduce).
  2. PE-transpose the [128, 24] mean columns; stage them to a DRAM scratch as
     fp16 centered at 0.5 (mean-of-1296-uniforms is 0.5 +- 0.01, and only
     differences matter downstream, so fp16 keeps ~1e-3 relative accuracy
     while halving gather traffic).
  3. Toeplitz gather DMAs (one per channel) materialize all 101 shifted
     copies of the padded mean rows; a 0-stride gather broadcasts the base.
  4. VectorE: fp16 in-place diff, |.| channel-reduce to fp32, out-of-range
     mask multiply -> window features [101, 1024] (+ ones row = bias trick).
  5. PE matmul [102]x[128 t]x[128 out] with fc weights (bias folded in as an
     extra contraction row), VectorE relu PSUM->SBUF, contiguous DMA out.
"""

import sys

if "/opt/trn_rl_repo" not in sys.path:
    sys.path.insert(0, "/opt/trn_rl_repo")

import numpy as np

N_CORES = 8
B, T, H, W, C = 16, 1024, 27, 48, 3
S = H * W                 # 1296 spatial positions
ROW = S * C               # 3888 floats per frame
LW = 101                  # lookup window
PAD = 50
OD = 128                  # output dim
BPC = B // N_CORES        # batches per core = 2
PADROW = T + LW - 1       # 1124
FT = T // 128             # 8 frame-tiles per batch
XCH = 2                   # chunk-DMAs per x tile
CENTER = 0.5              # mean centering before the fp16 staging cast
# The runtime picks the DMA engine from the DRAM-side address granule, so a
# gather whose reads all land in one ~13 KB region serializes onto a single
# engine (~24 GB/s). Stage the mean rows into NREP replicas spaced REPS
# elements apart and split each gather into w-chunks reading distinct
# replicas so the chunks land on distinct engines. The stride is an odd
# multiple of 2/4/8 KB so replicas stay distinct mod 16 for any granule size.
NREP = 4
REPS = 70656              # fp16 elements between replicas (141312 B)
# window rows are PERMUTED so that dest row 0 is w=50 (the base row): the
# on-chip base broadcast can then read partition 0 of the gather tile with no
# extra row DMA. fc weights and the mask are row-permuted on the host to
# match, so the matmul contraction is unchanged.
PERM = [50] + [w for w in range(LW) if w != 50]
# (dest_row0, dest_row1, src_w0, replica)
WCHUNKS = [(0, 1, 50, 3), (1, 26, 0, 0), (26, 51, 25, 1),
           (51, 76, 51, 2), (76, 101, 76, 3)]

_CACHE = {}


def _build_program():
    import concourse.bass as bass
    import concourse.tile as tile
    from concourse import bacc, mybir
    from concourse.ap import AP

    f32 = mybir.dt.float32
    f16 = mybir.dt.float16
    bf16 = mybir.dt.bfloat16
    nc = bacc.Bacc("TRN2", target_bir_lowering=False, debug=False)

    xs = nc.dram_tensor("xs", [BPC * T, ROW], f32, kind="ExternalInput")
    fcwb = nc.dram_tensor("fcwb", [LW + 1, OD], f32, kind="ExternalInput")
    maskw = nc.dram_tensor("maskw", [LW, T], bf16, kind="ExternalInput")
    ident = nc.dram_tensor("ident", [128, 128], f32, kind="ExternalInput")
    y = nc.dram_tensor("y", [BPC * T, OD], f32, kind="ExternalOutput")
    mcpad = nc.dram_tensor("mcpad", [NREP * REPS], f16)
    mc_ap = mcpad[:]

    def mc_view(offset, dims):
        return AP(tensor=mc_ap.tensor, offset=offset, ap=tuple(dims))

    with tile.TileContext(nc) as tc:
        with (
            tc.tile_pool(name="consts", bufs=1) as consts,
            tc.tile_pool(name="xin", bufs=9) as xin,
            tc.tile_pool(name="junk", bufs=1) as junkp,
            tc.tile_pool(name="sums", bufs=2) as sumsp,
            tc.tile_pool(name="stg", bufs=2) as stgp,
            tc.tile_pool(name="gath", bufs=2) as gathp,
            tc.tile_pool(name="wf", bufs=2) as wfp,
            tc.tile_pool(name="outs", bufs=4) as outsp,
            tc.tile_pool(name="zrow", bufs=1) as zrowp,
            tc.tile_pool(name="pst", bufs=2, space="PSUM") as pst,
            tc.tile_pool(name="pso", bufs=4, space="PSUM") as pso,
        ):
            fcwb_sb = consts.tile([LW + 1, OD], f32)
            nc.gpsimd.dma_start(fcwb_sb[:], fcwb[:])
            maskw_sb = consts.tile([LW, T], bf16)
            nc.gpsimd.dma_start(maskw_sb[:], maskw[:])
            ident_sb = consts.tile([128, 128], f32)
            nc.gpsimd.dma_start(ident_sb[:], ident[:])

            # zero-fill the used region of each replica (the padded wings
            # must read as 0.0; the inter-replica gaps are never read)
            z = zrowp.tile([24, PADROW], f16)
            nc.vector.memset(z[:], 0.0)
            nc.gpsimd.dma_start(
                mc_view(0, [(REPS, NREP), (1, BPC * C * PADROW)]), z[:]
            )

            # all x-chunk DMAs are emitted FIRST (no input deps) and split
            # across BOTH HWDGE rings (sync + scalar): each ring's ~1.8us
            # per-instruction descriptor-gen is serial, so two rings halve
            # the issue ramp. Emitting them before any compute keeps every
            # ring's FIFO free of wait-inversions.
            xts = []
            for b in range(BPC):
                for i in range(FT):
                    xt = xin.tile([128, ROW], f32)
                    xts.append(xt)
                    for k in range(XCH):
                        lo, hi = k * (ROW // XCH), (k + 1) * (ROW // XCH)
                        eng = nc.sync if k % 2 == 0 else nc.scalar
                        eng.dma_start(
                            xt[:, lo:hi],
                            xs[b * T + i * 128 : b * T + (i + 1) * 128, lo:hi],
                        )

            # -------- emission helpers (order = per-engine priority) --------
            sums_t, stg_t, sh_t, ba_t, wf_t = {}, {}, {}, {}, {}

            def stageA_tile(b, i):
                # channel 0 on ScalarE (activation accum), channels 1 and 2
                # on VectorE (strided reduce) - balances the two engines
                # under the ~96us stream time
                sums = sums_t[b]
                xv = xts[b * FT + i][:].rearrange("p (s c) -> p c s", c=C)
                junk = junkp.tile([128, S], f32)
                nc.scalar.activation(
                    junk[:],
                    xv[:, 0, :],
                    mybir.ActivationFunctionType.Copy,
                    bias=0.0,
                    scale=1.0 / S,
                    accum_out=sums[:, i : i + 1],
                )
                for c in (1, 2):
                    nc.vector.tensor_reduce(
                        sums[:, c * FT + i : c * FT + i + 1],
                        xv[:, c, :],
                        axis=mybir.AxisListType.X,
                        op=mybir.AluOpType.add,
                    )

            def stageA_finish(b):
                # normalize the raw VectorE sums, transpose [128, 24] ->
                # [24, 128], center+cast to fp16, stage to the DRAM replicas
                sums = sums_t[b]
                nc.vector.tensor_scalar_mul(
                    sums[:, FT : C * FT], sums[:, FT : C * FT], 1.0 / S
                )
                ps = pst.tile([C * FT, 128], f32)
                nc.tensor.transpose(ps[:], sums[:], ident_sb[:])
                stg = stgp.tile([C * FT, 128], f16)
                nc.vector.tensor_scalar_sub(stg[:], ps[:], CENTER)
                for r in range(NREP):
                    nc.gpsimd.dma_start(
                        mc_view(
                            r * REPS + b * C * PADROW + PAD,
                            [(PADROW, C), (128, FT), (1, 128)],
                        ),
                        stg[:],
                    )

            def stageB_gather(b):
                # gathers on the scalar HWDGE ring (sync carries the stream);
                # base row (= permuted row 0 of sh) replicated on gpsimd
                sh = gathp.tile([LW, C * T], f16, tag="sh")
                ba = gathp.tile([LW, C * T], f16, tag="ba")
                sh_t[b], ba_t[b] = sh, ba
                for r0, r1, w0, rep in WCHUNKS:
                    nc.scalar.dma_start(
                        sh[r0:r1, :],
                        mc_view(
                            rep * REPS + b * C * PADROW + w0,
                            [(1, r1 - r0), (PADROW, C), (1, T)],
                        ),
                    )
                nc.gpsimd.partition_broadcast(ba[:], sh[0:1, :], channels=LW)

            def stageB_dist(b, sub_eng):
                # |sh - ba| channel-reduce + out-of-range mask -> wf.
                # For batch 0 the subtract runs on gpsimd so VectorE (busy
                # pacing batch 1 tile reduces mid-stream) never stalls.
                sh, ba = sh_t[b], ba_t[b]
                wf = wfp.tile([LW + 1, T], f32)
                wf_t[b] = wf
                # row LW must be 1.0 (bias trick); engines can only start at
                # partition 0/32/64/96: fill everything, overwrite rows 0..100
                sub_eng.memset(wf[:], 1.0)
                sub_eng.tensor_sub(sh[:], sh[:], ba[:])
                shv = sh[:].rearrange("p (c t) -> p t c", c=C)
                nc.vector.tensor_reduce(
                    wf[0:LW, :],
                    shv,
                    axis=mybir.AxisListType.X,
                    op=mybir.AluOpType.add,
                    apply_absolute_value=True,
                )
                nc.vector.tensor_mul(wf[0:LW, :], wf[0:LW, :], maskw_sb[:])

            def stageC(b):
                wf = wf_t[b]
                for j in range(FT):
                    po = pso.tile([128, OD], f32)
                    nc.tensor.matmul(po[:], wf[:, bass.ts(j, 128)], fcwb_sb[:])
                    osb = outsp.tile([128, OD], f32)
                    nc.vector.tensor_scalar_max(osb[:], po[:], 0.0)
                    # batch 0 outputs go mid-stream on the idle gpsimd ring;
                    # batch 1 outputs are on the tail - split across the
                    # sync+scalar rings (idle by then) to halve issue time
                    if b == 0:
                        eng = nc.gpsimd
                    else:
                        eng = nc.sync if j % 2 == 0 else nc.scalar
                    eng.dma_start(
                        y[b * T + j * 128 : b * T + (j + 1) * 128, :], osb[:]
                    )

            # -------- emission schedule --------
            # b0's stage B/C is woven between b1's tile reduces so its
            # windowed distances + matmuls overlap the second half of the
            # x-stream without stalling any engine's FIFO.
            for b in range(BPC):
                sums_t[b] = sumsp.tile(
                    [128, C * FT], f32, name="sums", tag="sums"
                )
            for i in range(FT):
                stageA_tile(0, i)
            stageA_finish(0)
            stageB_gather(0)
            for i in range(0, 4):
                stageA_tile(1, i)
            stageB_dist(0, nc.gpsimd)
            stageC(0)
            for i in range(4, FT):
                stageA_tile(1, i)
            stageA_finish(1)
            stageB_gather(1)
            stageB_dist(1, nc.vector)
            stageC(1)

    nc.compile()
    return nc


def get_nc():
    if "nc" not in _CACHE:
        _CACHE["nc"] = _build_program()
    return _CACHE["nc"]


def make_host_inputs(x, fc_w, fc_b):
    """Per-core input maps from the full problem inputs."""
    x = np.ascontiguousarray(x, dtype=np.float32).reshape(B, T, ROW)
    wT = fc_w.T.astype(np.float32)[PERM]          # window-row permutation
    fcwb = np.concatenate([wT, fc_b[None, :].astype(np.float32)], axis=0)
    fcwb = np.ascontiguousarray(fcwb)
    u = np.arange(T)[None, :] + np.arange(LW)[:, None] - PAD
    import ml_dtypes

    maskw = ((u >= 0) & (u < T)).astype(ml_dtypes.bfloat16)[PERM]
    maskw = np.ascontiguousarray(maskw)
    ident = np.eye(128, dtype=np.float32)
    in_maps = []
    for ci in range(N_CORES):
        shard = np.ascontiguousarray(
            x[ci * BPC : (ci + 1) * BPC].reshape(BPC * T, ROW)
        )
        in_maps.append(
            {"xs": shard, "fcwb": fcwb, "maskw": maskw, "ident": ident}
        )
    return in_maps


def kernel(x, fc_w, fc_b):
    from concourse.bass_utils import run_bass_kernel_spmd

    nc = get_nc()
    in_maps = make_host_inputs(x, fc_w, fc_b)
    res = run_bass_kernel_spmd(nc, in_maps, list(range(N_CORES)))
    outs = [r["y"].reshape(BPC, T, OD) for r in res.results]
    return np.concatenate(outs, axis=0).astype(np.float32)



# revision 6
# speedup vs baseline: 1.9286x; 1.9286x over previous
"""Trainium2 Bass kernel for nn_ColorHistograms (v2).

Per NeuronCore (2 batch elements, 8 cores):
  x is quantized to uint8 on the host (rel-err ~2e-3 << 2e-2 gate), which
  cuts the HBM stream from 32MB f32 to 8MB u8 per core.
  - channel 0 streams as plain u8 tiles [128 frames, 1296]; spatial means
    via DVE tensor_reduce (even tiles) + ScalarE activation-accum (odd).
  - channels 1,2 stream via SWDGE cast-DMA (u8 -> fp16, write-side bound
    ~400B/ns) in [s-partition, t] layout; spatial means via PE:
    LDWEIGHTS(x-chunk [128s x 128t]) + 1-col ones matmul accumulating in
    PSUM -- sustained 26.7ns per 16K elements (5x DVE reduce rate).
  Means [128, 24] -> PE transpose -> scale+center -> fp16 -> staged to a
  DRAM scratch in NREP replicas (spreads the Toeplitz gather across SDMA
  engines, which are picked by DRAM address granule).
  Toeplitz gather materializes 101 shifted copies [101, 3c x 1024t]; the
  t-aligned base row is broadcast to 101 partitions via a PE ones-matmul
  into PSUM (1.3us vs 4.7us gpsimd partition_broadcast); DVE subtract,
  abs-channel-reduce, out-of-range mask, then fp16 matmuls with fc
  weights (bias folded as an extra contraction row) + ReLU + store.
"""

import sys

if "/opt/trn_rl_repo" not in sys.path:
    sys.path.insert(0, "/opt/trn_rl_repo")

import numpy as np

N_CORES = 8
B, T, H, W, C = 16, 1024, 27, 48, 3
S = H * W                 # 1296 spatial positions
LW = 101
PAD = 50
OD = 128
BPC = B // N_CORES        # 2 batches per core
PADROW = T + LW - 1       # 1124
FT = T // 128             # 8 frame tiles per batch
SCALE = 1.0 / (255.0 * S)  # u8 sums -> mean
CENTER = 0.5
# PE-mean s-chunking: s = j*128 + p for j<10, tail chunk j=10 has 16 rows
SJ = 10                   # full 128-row chunks
STAIL = S - SJ * 128      # 16
# DRAM mean-scratch replicas (gather engine spread)
NREP = 8
REPS = 70656              # fp16 elements between replicas
# gather w-chunks: 8 chunks of 13/13/13/13/13/13/13/10 rows
GCH = [(0, 13), (13, 26), (26, 39), (39, 52), (52, 65), (65, 78),
       (78, 91), (91, 101)]

_CACHE = {}


def _build_program():
    import concourse.bass as bass
    import concourse.tile as tile
    from concourse import bacc, mybir
    from concourse.ap import AP

    f32 = mybir.dt.float32
    f16 = mybir.dt.float16
    bf16 = mybir.dt.bfloat16
    u8 = mybir.dt.uint8
    nc = bacc.Bacc("TRN2", target_bir_lowering=False, debug=False)

    # c0 plane: [b, t, s] u8 rows
    x0 = nc.dram_tensor("x0", [BPC * T, S], u8, kind="ExternalInput")
    # c1,c2 planes in [b, c', p, j*t] u8: rows = ((b*2 + c')*128 + p)
    x12 = nc.dram_tensor("x12", [BPC * 2 * 128, SJ * T], u8,
                         kind="ExternalInput")
    # s-tail rows (j=10): [b, c', p<16, t]
    x12t = nc.dram_tensor("x12t", [BPC * 2 * STAIL, T], u8,
                          kind="ExternalInput")
    fcwb = nc.dram_tensor("fcwb", [LW + 1, OD], f16, kind="ExternalInput")
    maskw = nc.dram_tensor("maskw", [LW, T], bf16, kind="ExternalInput")
    ident = nc.dram_tensor("ident", [128, 128], f32, kind="ExternalInput")
    y = nc.dram_tensor("y", [BPC * T, OD], f32, kind="ExternalOutput")
    mcpad = nc.dram_tensor("mcpad", [NREP * REPS], f16)
    mc_ap = mcpad[:]

    def mc_view(offset, dims):
        return AP(tensor=mc_ap.tensor, offset=offset, ap=tuple(dims))

    with tile.TileContext(nc) as tc:
        with (
            tc.tile_pool(name="consts", bufs=1) as consts,
            tc.tile_pool(name="x0in", bufs=8) as x0in,
            tc.tile_pool(name="cast", bufs=4) as castp,
            tc.tile_pool(name="ctail", bufs=4) as ctailp,
            tc.tile_pool(name="sums", bufs=2) as sumsp,
            tc.tile_pool(name="stg", bufs=2) as stgp,
            tc.tile_pool(name="mrow", bufs=2) as mrowp,
            tc.tile_pool(name="gath", bufs=2) as gathp,
            tc.tile_pool(name="wf", bufs=2) as wfp,
            tc.tile_pool(name="outs", bufs=4) as outsp,
            tc.tile_pool(name="junk", bufs=1) as junkp,
            tc.tile_pool(name="zrow", bufs=1) as zrowp,
            tc.tile_pool(name="pmean", bufs=2, space="PSUM") as pmean,
            tc.tile_pool(name="pba", bufs=2, space="PSUM") as pba,
            tc.tile_pool(name="ppt", bufs=1, space="PSUM") as ppt,
            tc.tile_pool(name="pout", bufs=2, space="PSUM") as pout,
        ):
            fcwb_sb = consts.tile([LW + 1, OD], f16)
            nc.gpsimd.dma_start(fcwb_sb[:], fcwb[:])
            maskw_sb = consts.tile([LW, T], bf16)
            nc.gpsimd.dma_start(maskw_sb[:], maskw[:])
            ident_sb = consts.tile([128, 128], f32)
            nc.gpsimd.dma_start(ident_sb[:], ident[:])
            ones_sb = consts.tile([128, 1], f16)
            nc.vector.memset(ones_sb[:], 1.0)
            onesw_sb = consts.tile([1, LW], f16)
            nc.vector.memset(onesw_sb[:], 1.0)

            # zero the used regions of all replicas (padded wings read 0)
            z = zrowp.tile([48, PADROW], f16)
            nc.vector.memset(z[:], 0.0)
            nc.gpsimd.dma_start(
                mc_view(0, [(REPS, NREP), (1, BPC * C * PADROW)]), z[:]
            )

            # ---- x stream: emit all DMAs up front ----
            # c0 plain tiles alternate sync/scalar rings; c1/c2 cast halves
            # on the gpsimd (SWDGE) ring.
            x0t, cst, cstl = {}, {}, {}
            for b in range(BPC):
                for i in range(FT):
                    xt = x0in.tile([128, S], u8, name="x0t")
                    x0t[(b, i)] = xt
                    eng = nc.sync if i % 2 == 0 else nc.scalar
                    eng.dma_start(
                        xt[:], x0[b * T + i * 128: b * T + (i + 1) * 128, :]
                    )
                for cc in range(2):
                    ct = castp.tile([128, SJ * T], f16, name="cslab")
                    cst[(b, cc)] = ct
                    row0 = (b * 2 + cc) * 128
                    for h in range(2):
                        # t-halves so PE can start at half-slab
                        nc.gpsimd.dma_start(
                            ct[:].rearrange("p (j t) -> p j t", j=SJ)[
                                :, :, h * 512:(h + 1) * 512],
                            AP(tensor=x12[:].tensor,
                               offset=row0 * SJ * T + h * 512,
                               ap=((SJ * T, 128), (T, SJ), (1, 512))),
                        )
                    ctl = ctailp.tile([STAIL, T], f16, name="ctail")
                    cstl[(b, cc)] = ctl
                    trow0 = (b * 2 + cc) * STAIL
                    nc.gpsimd.dma_start(
                        ctl[:], x12t[trow0: trow0 + STAIL, :]
                    )

            sums_t, stg_t, sh_t, mrow_t = {}, {}, {}, {}

            def c0_means(b, i):
                # plain u8 tile -> per-frame sums; even tiles DVE, odd ACT
                sums = sums_t[b]
                xt = x0t[(b, i)]
                if i % 2 == 0:
                    nc.vector.tensor_reduce(
                        sums[:, i:i + 1], xt[:],
                        axis=mybir.AxisListType.X, op=mybir.AluOpType.add)
                else:
                    jk = junkp.tile([128, S], f32)
                    nc.scalar.activation(
                        jk[:], xt[:], mybir.ActivationFunctionType.Copy,
                        bias=0.0, scale=1.0, accum_out=sums[:, i:i + 1])

            def pe_means(b, cc, half):
                # cast slab half -> 8 t-tile accumulation groups on PE
                ct = cst[(b, cc)]
                ctv = ct[:].rearrange("p (j t) -> p j t", j=SJ)
                ctl = cstl[(b, cc)]
                pm = pmean_t[b]
                for jt in range(4 * half, 4 * (half + 1)):
                    col = (1 + cc) * FT + jt
                    for j in range(SJ):
                        nc.tensor.matmul(
                            pm[:, col:col + 1],
                            ctv[:, j, jt * 128:(jt + 1) * 128],
                            ones_sb[:],
                            start=(j == 0), stop=False)
                    nc.tensor.matmul(
                        pm[:, col:col + 1],
                        ctl[:, jt * 128:(jt + 1) * 128],
                        ones_sb[0:STAIL, :],
                        start=False, stop=True)

            def finish_means(b):
                # PE mean cols -> sums SBUF; transpose; scale+center to f16;
                # stage to NREP DRAM replicas; load base row back
                sums = sums_t[b]
                pm = pmean_t[b]
                nc.vector.tensor_copy(sums[:, FT:C * FT], pm[:, FT:C * FT])
                pt = ppt.tile([C * FT, 128], f32, name="pt")
                nc.tensor.transpose(pt[:], sums[:], ident_sb[:])
                stg = stgp.tile([C * FT, 8 * 128], f16, name="stg")
                nc.vector.tensor_scalar(
                    out=stg[:, 0:128], in0=pt[:], scalar1=SCALE,
                    scalar2=CENTER, op0=mybir.AluOpType.mult,
                    op1=mybir.AluOpType.subtract)
                for d in (128, 256, 512):
                    nc.vector.tensor_copy(stg[:, d:2 * d], stg[:, 0:d])
                for c in range(C):
                    nc.gpsimd.dma_start(
                        mc_view(b * C * PADROW + c * PADROW + PAD,
                                [(128, FT), (REPS, NREP), (1, 128)]),
                        stg[c * FT:(c + 1) * FT, :].rearrange(
                            "p (r t) -> p r t", r=NREP),
                    )
                mrow = mrowp.tile([1, C * T], f16, name="mrow")
                mrow_t[b] = mrow
                nc.sync.dma_start(
                    mrow[:],
                    mc_view(b * C * PADROW + PAD, [(1, 1), (PADROW, C), (1, T)]),
                )

            def gather(b):
                sh = gathp.tile([LW, C * T], f16, name="sh", tag="sh")
                sh_t[b] = sh
                for k, (w0, w1) in enumerate(GCH):
                    eng = nc.scalar if k % 2 == 0 else nc.sync
                    eng.dma_start(
                        sh[w0:w1, :],
                        mc_view(k * REPS + b * C * PADROW + w0,
                                [(1, w1 - w0), (PADROW, C), (1, T)]),
                    )

            def dist(b):
                # PE: broadcast base row to 101 partitions per channel (PSUM),
                # DVE: subtract, abs-channel-reduce, mask
                sh = sh_t[b]
                mrow = mrow_t[b]
                wf = wfp.tile([LW + 1, T], f16, name="wf")
                nc.vector.memset(wf[:], 1.0)
                for c in range(C):
                    for hh in range(2):
                        lo = c * T + hh * 512
                        bap = pba.tile([LW, 512], f32, name="ba")
                        nc.tensor.matmul(
                            bap[:], onesw_sb[:], mrow[:, lo:lo + 512],
                            start=True, stop=True)
                        nc.vector.tensor_sub(
                            sh[:, lo:lo + 512], sh[:, lo:lo + 512], bap[:])
                with nc.allow_low_precision(reason="3-term abs sum in f16"):
                    nc.vector.tensor_reduce(
                        wf[0:LW, :], sh[:].rearrange("p (c t) -> p t c", c=C),
                        axis=mybir.AxisListType.X, op=mybir.AluOpType.add,
                        apply_absolute_value=True)
                nc.vector.tensor_mul(wf[0:LW, :], wf[0:LW, :], maskw_sb[:])
                return wf

            def matmuls(b, wf):
                for j in range(FT):
                    po = pout.tile([128, OD], f32, name="po")
                    nc.tensor.matmul(po[:], wf[:, bass.ts(j, 128)], fcwb_sb[:],
                                     start=True, stop=True)
                    osb = outsp.tile([128, OD], f32, name="osb")
                    nc.scalar.activation(
                        osb[:], po[:], mybir.ActivationFunctionType.Relu)
                    eng = nc.sync if j % 2 == 0 else nc.scalar
                    eng.dma_start(
                        y[b * T + j * 128: b * T + (j + 1) * 128, :], osb[:])

            # ---- emission schedule ----
            pmean_t = {}
            for b in range(BPC):
                sums_t[b] = sumsp.tile([128, C * FT], f32, name="sums")
                pmean_t[b] = pmean.tile([128, C * FT], f32, name="pm")
            # batch 0: means as tiles/slabs land
            for i in range(FT):
                c0_means(0, i)
            for cc in range(2):
                for h in range(2):
                    pe_means(0, cc, h)
            finish_means(0)
            gather(0)
            # batch 1 stream work interleaves here
            for i in range(FT):
                c0_means(1, i)
            wf0 = dist(0)
            matmuls(0, wf0)
            for cc in range(2):
                for h in range(2):
                    pe_means(1, cc, h)
            finish_means(1)
            gather(1)
            wf1 = dist(1)
            matmuls(1, wf1)

    nc.compile()
    return nc


def get_nc():
    if "nc" not in _CACHE:
        _CACHE["nc"] = _build_program()
    return _CACHE["nc"]


def make_host_inputs(x, fc_w, fc_b):
    import ml_dtypes

    xq = np.clip(np.rint(np.asarray(x, dtype=np.float32) * 255.0), 0, 255)
    xq = xq.astype(np.uint8).reshape(B, T, S, C)
    x0_all = np.ascontiguousarray(xq[:, :, :, 0])            # [B, T, S]
    # c1/c2: [B, c', p, j, t] with s = j*128 + p (j < 10), tail s >= 1280
    x12_all = xq[:, :, :SJ * 128, 1:3]                       # [B,T,1280,2]
    x12_all = x12_all.reshape(B, T, SJ, 128, 2)
    x12_all = np.ascontiguousarray(
        x12_all.transpose(0, 4, 3, 2, 1))                    # [B,2,128,SJ,T]
    x12t_all = np.ascontiguousarray(
        xq[:, :, SJ * 128:, 1:3].transpose(0, 3, 2, 1))      # [B,2,16,T]

    wT = fc_w.T.astype(np.float32)                           # [101, 128]
    fcwb = np.concatenate([wT, fc_b[None, :].astype(np.float32)], axis=0)
    fcwb = np.ascontiguousarray(fcwb.astype(np.float16))
    u = np.arange(T)[None, :] + np.arange(LW)[:, None] - PAD
    maskw = np.ascontiguousarray(((u >= 0) & (u < T)).astype(ml_dtypes.bfloat16))
    ident = np.eye(128, dtype=np.float32)
    in_maps = []
    for ci in range(N_CORES):
        b0 = ci * BPC
        in_maps.append({
            "x0": np.ascontiguousarray(
                x0_all[b0:b0 + BPC].reshape(BPC * T, S)),
            "x12": np.ascontiguousarray(
                x12_all[b0:b0 + BPC].reshape(BPC * 2 * 128, SJ * T)),
            "x12t": np.ascontiguousarray(
                x12t_all[b0:b0 + BPC].reshape(BPC * 2 * STAIL, T)),
            "fcwb": fcwb, "maskw": maskw, "ident": ident,
        })
    return in_maps


def kernel(x, fc_w, fc_b):
    from concourse.bass_utils import run_bass_kernel_spmd

    nc = get_nc()
    in_maps = make_host_inputs(x, fc_w, fc_b)
    res = run_bass_kernel_spmd(nc, in_maps, list(range(N_CORES)))
    outs = [r["y"].reshape(BPC, T, OD) for r in res.results]
    return np.concatenate(outs, axis=0).astype(np.float32)
